# revision 26
# baseline (speedup 1.0000x reference)
"""Trainium2 Bass kernel: AdaptiveDiscretizedNeuralODE (30-step scan with
training-mode BatchNorm over the HW=1024 channel axis, ReLU6, residual).

Design:
 - Channel-shard the 1024 BN channels over 8 NeuronCores -> 128 channels/core
   = the 128 SBUF partitions. BN stats/affine/ReLU6/residual are per-channel,
   so the 8 cores are fully independent (no collectives).
 - Scale folding: BN is invariant under per-layer positive rescaling of its
   input (eps adjusted by 1/alpha_l^2), so the recurrence
       y_{l+1} = (1-dt_l) y_l + dt_l relu6(BN(y_l + m_l*x1))
   becomes, with z_l = y_l/alpha_l + (m_l/alpha_l)*x1:
       z_{l+1} = z_l + min(relu(a_l z_l + b_l), 6 c_l) + (mtil_{l+1}-mtil_l)*x1
   where a_l, b_l fold the BN affine, gamma/beta, and c_l = dt_l/alpha_{l+1}.
 - The state z stays resident in PSUM ([128, 4096] fp32 = all 8 banks) for all
   30 steps; every add into the state is an identity-matmul accumulation on
   the otherwise-idle TensorEngine (PSUM accumulates in fp32 exactly).
 - Steady-state engine split per layer (~10.8 us, DVE and ACT both ~95% busy):
     ACT:  u = Relu(a*z + b)  (4x1024, reads PSUM, per-partition scale/bias)
           Square(z_new) accum -> SS  (2x2048, reads PSUM)
           sqrt for rstd
     DVE:  wp = min(u, 6c) + P_l -> bf16, accum -> sum(wp)   (fused STT)
           P_{l+1} = x1_bf16 * dmtil_{l+1} (bf16 2x-mode TTs, flat tables)
           small [128,1] stats ops (cheap tensor_scalar forms only)
     PE:   z += I @ wp  (8 bank matmuls, bf16 rhs, fp32 accumulate)
 - S (sum) is tracked via the wp accumulators (S += sum(wp)); SS is measured
   from PSUM each layer. The initial state is seeded as a hi+lo bf16 pair
   (near-fp32-exact). Per-layer scalars (6c_l, N^2 eps_l, alpha_L) are baked
   as immediates; the neuron compile cache makes that one-time per input set.
 - Host side does layout only: reshape/transpose of x into per-core shards,
   parameter-table construction from delta_t/matrices/gamma/beta, and the
   inverse layout transform on the output.
"""
import numpy as np
import ml_dtypes

B, C, H, W = 16, 256, 32, 32
HW = H * W
NL = 30
EPS = 1e-5
NCORES = 8
P = 128
FB = B * C           # 4096 free elements per partition
BANK = 512           # psum bank = 512 fp32
NBANK = FB // BANK   # 8
WCH = 1024           # w-pass chunk
ACH = 2048           # ACT pass chunk
GCH = 2048           # gpsimd chunk
NRED = float(FB)

_cached = {}


def _host_params(delta_t, matrices):
    dt = np.clip(delta_t.astype(np.float64), 0, 6)[:, 0]
    m = matrices.reshape(NL, C).astype(np.float64)
    alpha = np.concatenate([[1.0], np.cumprod(1.0 - dt)])
    mtil = m / alpha[:NL, None]
    cc = dt / alpha[1:]
    g0 = 1.0 + mtil[0]
    dmt = mtil[1:] - mtil[:-1]                     # [29, 256]
    gfin = 1.0 - alpha[NL] * mtil[NL - 1]
    epst = EPS / alpha[:NL] ** 2
    n2eps = NRED * NRED * epst
    sixc = 6.0 * cc
    return dt, alpha, mtil, cc, g0, dmt, gfin, n2eps, sixc


def _build_program(sixc, n2eps, alpha_l):
    import concourse.tile as tile
    from concourse import bacc, mybir

    f32 = mybir.dt.float32
    bf16 = mybir.dt.bfloat16
    Alu = mybir.AluOpType
    Act = mybir.ActivationFunctionType

    nc = bacc.Bacc("TRN2", target_bir_lowering=False, debug=False,
                   num_devices=NCORES)
    x1_d = nc.dram_tensor("x1", [P, FB], f32, kind="ExternalInput").ap()
    # 29 flat bf16 rows (dmt), each 256-pattern tiled to 1024
    mtb_d = nc.dram_tensor("mtb", [P, 29 * 1024], bf16, kind="ExternalInput").ap()
    # 2 flat f32 rows: g0, gfin
    mtf_d = nc.dram_tensor("mtf", [P, 2 * 1024], f32, kind="ExternalInput").ap()
    ctab_d = nc.dram_tensor("ctab", [P, 3 * NL], f32, kind="ExternalInput").ap()
    id_d = nc.dram_tensor("ident", [P, P], bf16, kind="ExternalInput").ap()
    out_d = nc.dram_tensor("out", [P, FB], f32, kind="ExternalOutput").ap()

    with tile.TileContext(nc) as tc:
        with (
            tc.tile_pool(name="big", bufs=1) as big,
            tc.tile_pool(name="upool", bufs=2) as upool,
            tc.tile_pool(name="jpool", bufs=2) as jpool,
            tc.tile_pool(name="wpool", bufs=2) as wpool,
            tc.tile_pool(name="apool", bufs=2) as apool,
            tc.tile_pool(name="spool", bufs=3) as spool,
            tc.tile_pool(name="dpool", bufs=3) as dpool,
            tc.tile_pool(name="ppro", bufs=2) as ppro,
            tc.tile_pool(name="pp", bufs=1, space="PSUM") as pp,
        ):
            x1t = big.tile([P, FB], f32, name="x1t")
            x1b = big.tile([P, FB], bf16, name="x1b")
            pbb = [big.tile([P, FB], bf16, name="pbb0"),
                   big.tile([P, FB], bf16, name="pbb1")]
            ct = big.tile([P, 3 * NL], f32, name="ct")
            tI = big.tile([P, P], bf16, name="tI")
            g0r = big.tile([P, 1024], f32, name="g0r")
            gfr = big.tile([P, 1024], f32, name="gfr")
            pfin = big.tile([P, FB], f32, name="pfin")
            zp = pp.tile([P, FB], f32, name="zp")

            def sl(i, w):
                return slice(i * w, (i + 1) * w)

            # ---- input DMAs
            for chi in range(4):
                nc.sync.dma_start(x1t[:, sl(chi, 1024)], x1_d[:, sl(chi, 1024)])
            nc.sync.dma_start(ct[:], ctab_d)
            nc.sync.dma_start(tI[:], id_d)
            nc.sync.dma_start(g0r[:], mtf_d[:, 0:1024])
            nc.sync.dma_start(gfr[:], mtf_d[:, 1024:2048])
            mrow = dpool.tile([P, 1024], bf16, name="mrow_p0", tag="dmb")
            nc.sync.dma_start(mrow[:], mtb_d[:, 0:1024])

            # ---- prologue: per-chunk pipeline  x1 -> z0 -> (zhi, zlo) -> PE
            Sacc0 = apool.tile([P, 4], f32, name="Sacc_p", tag="Sacc4")
            for chi in range(4):
                z0c = ppro.tile([P, 1024], f32, name=f"z0_{chi}", tag="z0")
                nc.vector.scalar_tensor_tensor(z0c[:], x1t[:, sl(chi, 1024)],
                                               0.0, g0r[:], op0=Alu.bypass,
                                               op1=Alu.mult,
                                               accum_out=Sacc0[:, chi:chi + 1])
                zhic = ppro.tile([P, 1024], bf16, name=f"zhi{chi}", tag="zh")
                nc.vector.tensor_copy(zhic[:], z0c[:])
                zloc = ppro.tile([P, 1024], bf16, name=f"zlo{chi}", tag="zl")
                nc.vector.scalar_tensor_tensor(zloc[:], z0c[:], 0.0, zhic[:],
                                               op0=Alu.bypass, op1=Alu.subtract)
                for b2 in range(2):
                    b = 2 * chi + b2
                    nc.tensor.matmul(zp[:, sl(b, BANK)], tI[:],
                                     zhic[:, sl(b2, BANK)], start=True, stop=True)
                    nc.tensor.matmul(zp[:, sl(b, BANK)], tI[:],
                                     zloc[:, sl(b2, BANK)], start=False, stop=True)
                # (hi+lo seed: near-fp32-exact initial state)
            SSacc = apool.tile([P, 2], f32, name="SSacc_p", tag="SSacc")
            for q in range(2):
                jt = jpool.tile([P, ACH], f32, name=f"j_p{q}", tag="junk")
                nc.scalar.activation(jt[:], zp[:, sl(q, ACH)], Act.Square,
                                     bias=0.0, scale=1.0,
                                     accum_out=SSacc[:, q:q + 1])
            nc.vector.tensor_copy(x1b[:], x1t[:])
            for g in range(4):
                nc.vector.tensor_tensor(pbb[0][:, sl(g, 1024)],
                                        x1b[:, sl(g, 1024)],
                                        mrow[:], op=Alu.mult)

            Scur = None
            s2e = None
            san = None
            for l in range(NL):
                # ---- per-layer BN coefficients
                # DVE head: SSg -> t1 -> v -> rc ; ACT tail: rs -> a/an2 -> bb
                if l == 0:
                    Scur = spool.tile([P, 1], f32, name="S0", tag="S")
                    nc.vector.tensor_reduce(Scur[:], Sacc0[:],
                                            axis=mybir.AxisListType.X, op=Alu.add)
                    s2e = spool.tile([P, 1], f32, name="s2e0", tag="s2e")
                    nc.vector.tensor_scalar(s2e[:], Scur[:], Scur[:],
                                            -float(n2eps[0]),
                                            op0=Alu.mult, op1=Alu.add)
                    san = spool.tile([P, 1], f32, name="san0", tag="san")
                    nc.vector.tensor_scalar(san[:], Scur[:], ct[:, NL:NL + 1],
                                            None, op0=Alu.mult)
                # chain: SSg -> v -> rc -> sqrt -> {a, bb}
                SSg = spool.tile([P, 1], f32, name=f"SSg{l}", tag="SSg")
                nc.vector.tensor_reduce(SSg[:], SSacc[:],
                                        axis=mybir.AxisListType.X, op=Alu.add)
                v = spool.tile([P, 1], f32, name=f"v{l}", tag="v")
                nc.vector.tensor_scalar(v[:], SSg[:], NRED, s2e[:],
                                        op0=Alu.mult, op1=Alu.subtract)
                rc = spool.tile([P, 1], f32, name=f"rc{l}", tag="rc")
                nc.vector.reciprocal(rc[:], v[:])
                rs = spool.tile([P, 1], f32, name=f"rs{l}", tag="rs")
                nc.scalar.activation(rs[:], rc[:], Act.Sqrt)
                a = spool.tile([P, 1], f32, name=f"a{l}", tag="a")
                nc.vector.tensor_scalar(a[:], rs[:], ct[:, l:l + 1], None,
                                        op0=Alu.mult)
                bb = spool.tile([P, 1], f32, name=f"bb{l}", tag="bb")
                nc.vector.tensor_scalar(bb[:], rs[:], san[:],
                                        ct[:, 2 * NL + l:2 * NL + l + 1],
                                        op0=Alu.mult, op1=Alu.add)
                # produce P_{l+1} on DVE (bf16 2x mode) while ACT runs A
                if l < NL - 2:
                    mrow = dpool.tile([P, 1024], bf16, name=f"mr{l}", tag="dmb")
                    nc.sync.dma_start(mrow[:], mtb_d[:, sl(l + 1, 1024)])
                    for g in range(4):
                        nc.vector.tensor_tensor(pbb[(l + 1) % 2][:, sl(g, 1024)],
                                                x1b[:, sl(g, 1024)],
                                                mrow[:], op=Alu.mult)

                if NL - 4 <= l:
                    g = l - (NL - 4)
                    nc.vector.tensor_tensor(pfin[:, sl(g, 1024)],
                                            x1t[:, sl(g, 1024)],
                                            gfr[:], op=Alu.mult)
                # ---- A: u = Relu(a*z + b) from PSUM (4 x 1024)
                us = []
                for q in range(4):
                    u = upool.tile([P, 1024], f32, name=f"u{l}_{q}", tag=f"u{q}")
                    nc.scalar.activation(u[:], zp[:, sl(q, 1024)], Act.Relu,
                                         bias=bb[:], scale=a[:])
                    us.append(u)
                # ---- wp = min(u, 6c) + P_l -> bf16 (accum -> sum), then PE adds
                WPC = [(0, 512), (512, 512), (1024, 1024), (2048, 1024),
                       (3072, 512), (3584, 512)]
                Wacc = apool.tile([P, len(WPC)], f32, name=f"Wacc{l}", tag="Wacc")
                for ch, (off, wid) in enumerate(WPC):
                    wb = wpool.tile([P, wid], bf16, name=f"w{l}_{ch}", tag=f"w{ch}")
                    uin = us[off // 1024][:, off % 1024:off % 1024 + wid]
                    if l < NL - 1:
                        nc.vector.scalar_tensor_tensor(
                            wb[:], uin, float(sixc[l]),
                            pbb[l % 2][:, off:off + wid],
                            op0=Alu.min, op1=Alu.add,
                            accum_out=Wacc[:, ch:ch + 1])
                    else:
                        nc.vector.tensor_scalar(wb[:], uin, float(sixc[l]), 0.0,
                                                op0=Alu.min, op1=Alu.add)
                    for b2 in range(wid // BANK):
                        b = (off + b2 * BANK) // BANK
                        nc.tensor.matmul(zp[:, sl(b, BANK)], tI[:],
                                         wb[:, sl(b2, BANK)],
                                         start=False, stop=True)
                if l < NL - 1:
                    # ---- Q: SS of new state, chunks trail the PE pipeline
                    QC = [(0, 2048), (2048, 2048)]
                    SSacc = apool.tile([P, len(QC)], f32, name=f"SSacc{l}",
                                       tag="SSacc")
                    for qi, (off, wid) in enumerate(QC):
                        jt = jpool.tile([P, wid], f32, name=f"j{l}_{qi}",
                                        tag=f"junk{qi}")
                        nc.scalar.activation(jt[:], zp[:, off:off + wid],
                                             Act.Square, bias=0.0, scale=1.0,
                                             accum_out=SSacc[:, qi:qi + 1])
                    # ---- S tracking (off critical path): S += sum(wp)
                    Wsum = spool.tile([P, 1], f32, name=f"Ws{l}", tag="Ws")
                    nc.vector.tensor_reduce(Wsum[:], Wacc[:],
                                            axis=mybir.AxisListType.X, op=Alu.add)
                    Snew = spool.tile([P, 1], f32, name=f"S{l + 1}", tag="S")
                    nc.vector.tensor_scalar(Snew[:], Wsum[:], Scur[:], None,
                                            op0=Alu.add)
                    Scur = Snew
                    s2e = spool.tile([P, 1], f32, name=f"s2e{l + 1}", tag="s2e")
                    nc.vector.tensor_scalar(s2e[:], Snew[:], Snew[:],
                                            -float(n2eps[l + 1]),
                                            op0=Alu.mult, op1=Alu.add)
                    san = spool.tile([P, 1], f32, name=f"san{l + 1}", tag="san")
                    nc.vector.tensor_scalar(san[:], Snew[:],
                                            ct[:, NL + l + 1:NL + l + 2],
                                            None, op0=Alu.mult)

            # ---- epilogue: out = alpha_L * z + gfin * x1
            for chi in range(4):
                o = upool.tile([P, 1024], f32, name=f"o{chi}", tag="obuf")
                nc.vector.scalar_tensor_tensor(o[:], zp[:, sl(chi, 1024)],
                                               float(alpha_l),
                                               pfin[:, sl(chi, 1024)],
                                               op0=Alu.mult, op1=Alu.add)
                nc.sync.dma_start(out_d[:, sl(chi, 1024)], o[:])

    nc.compile()
    return nc


def _get_nc(sixc, n2eps, alpha_l):
    key = (tuple(np.asarray(sixc, np.float64)),
           tuple(np.asarray(n2eps, np.float64)), float(alpha_l))
    if key not in _cached:
        _cached[key] = _build_program(sixc, n2eps, alpha_l)
    return _cached[key]


def _prepare_in_maps(x, delta_t, matrices, gamma, beta):
    dt, alpha, mtil, cc, g0, dmt, gfin, n2eps, sixc = _host_params(delta_t, matrices)

    reps = 1024 // C
    mtb = np.tile(dmt.astype(np.float32), (1, reps)).reshape(1, 29 * 1024)
    mtb_b = np.broadcast_to(mtb.astype(ml_dtypes.bfloat16), (P, 29 * 1024)).copy()
    mtf = np.tile(np.stack([g0, gfin]).astype(np.float32), (1, reps)).reshape(1, 2 * 1024)
    mtf_b = np.broadcast_to(mtf.astype(np.float32), (P, 2 * 1024)).copy()
    ident = np.eye(P, dtype=ml_dtypes.bfloat16)

    g64 = gamma.astype(np.float64)
    b64 = beta.astype(np.float64)
    x1_full = x.reshape(B, C, HW).transpose(2, 0, 1)   # [HW, B, C]

    in_maps = []
    for k in range(NCORES):
        slc = slice(k * P, (k + 1) * P)
        cgN = (cc[:, None] * g64[None, slc] * NRED).T.astype(np.float32)
        cgneg = (-cc[:, None] * g64[None, slc]).T.astype(np.float32)
        cb = (cc[:, None] * b64[None, slc]).T.astype(np.float32)
        ctab = np.ascontiguousarray(np.concatenate([cgN, cgneg, cb], axis=1))
        x1s = np.ascontiguousarray(x1_full[slc]).reshape(P, FB).astype(np.float32)
        in_maps.append({"x1": x1s, "mtb": mtb_b, "mtf": mtf_b, "ctab": ctab,
                        "ident": ident})
    return in_maps, (sixc, n2eps, alpha[NL])


def _gather(results):
    out = np.empty((HW, B, C), dtype=np.float32)
    for k in range(NCORES):
        out[k * P:(k + 1) * P] = results[k]["out"].reshape(P, B, C)
    return np.ascontiguousarray(out.transpose(1, 2, 0).reshape(B, C, H, W))


def _run(trace, **inputs):
    from concourse.bass_utils import run_bass_kernel_spmd
    in_maps, (sixc, n2eps, alpha_l) = _prepare_in_maps(
        np.asarray(inputs["x"]), np.asarray(inputs["delta_t"]),
        np.asarray(inputs["matrices"]), np.asarray(inputs["gamma"]),
        np.asarray(inputs["beta"]))
    nc = _get_nc(sixc, n2eps, alpha_l)
    res = run_bass_kernel_spmd(nc, in_maps, core_ids=list(range(NCORES)),
                               trace=trace)
    return _gather(res.results), res


def kernel(**inputs) -> np.ndarray:
    out, _ = _run(False, **inputs)
    return out


def kernel_traced(**inputs):
    """Returns (output, BassKernelResults) with exec_time_ns populated."""
    return _run(True, **inputs)


# revision 27
# speedup vs baseline: 1.0061x; 1.0061x over previous
"""Trainium2 Bass kernel: AdaptiveDiscretizedNeuralODE (30-step scan with
training-mode BatchNorm over the HW=1024 channel axis, ReLU6, residual).

Design:
 - Channel-shard the 1024 BN channels over 8 NeuronCores -> 128 channels/core
   = the 128 SBUF partitions. BN stats/affine/ReLU6/residual are per-channel,
   so the 8 cores are fully independent (no collectives).
 - Scale folding: BN is invariant under per-layer positive rescaling of its
   input (eps adjusted by 1/alpha_l^2), so the recurrence
       y_{l+1} = (1-dt_l) y_l + dt_l relu6(BN(y_l + m_l*x1))
   becomes, with z_l = y_l/alpha_l + (m_l/alpha_l)*x1:
       z_{l+1} = z_l + min(relu(a_l z_l + b_l), 6 c_l) + (mtil_{l+1}-mtil_l)*x1
   where a_l, b_l fold the BN affine, gamma/beta, and c_l = dt_l/alpha_{l+1}.
 - The state z stays resident in PSUM ([128, 4096] fp32 = all 8 banks) for all
   30 steps; every add into the state is an identity-matmul accumulation on
   the otherwise-idle TensorEngine (PSUM accumulates in fp32 exactly).
 - Steady-state engine split per layer (~10.8 us, DVE and ACT both ~95% busy):
     ACT:  u = Relu(a*z + b)  (4x1024, reads PSUM, per-partition scale/bias)
           Square(z_new) accum -> SS  (2x2048, reads PSUM)
           sqrt for rstd
     DVE:  wp = min(u, 6c) + P_l -> bf16, accum -> sum(wp)   (fused STT)
           P_{l+1} = x1_bf16 * dmtil_{l+1} (bf16 2x-mode TTs, flat tables)
           small [128,1] stats ops (cheap tensor_scalar forms only)
     PE:   z += I @ wp  (8 bank matmuls, bf16 rhs, fp32 accumulate)
 - S (sum) is tracked via the wp accumulators (S += sum(wp)); SS is measured
   from PSUM each layer. The initial state is seeded as a hi+lo bf16 pair
   (near-fp32-exact). Per-layer scalars (6c_l, N^2 eps_l, alpha_L) are baked
   as immediates; the neuron compile cache makes that one-time per input set.
 - Host side does layout only: reshape/transpose of x into per-core shards,
   parameter-table construction from delta_t/matrices/gamma/beta, and the
   inverse layout transform on the output.
"""
import numpy as np
import ml_dtypes

B, C, H, W = 16, 256, 32, 32
HW = H * W
NL = 30
EPS = 1e-5
NCORES = 8
P = 128
FB = B * C           # 4096 free elements per partition
BANK = 512           # psum bank = 512 fp32
NBANK = FB // BANK   # 8
WCH = 1024           # w-pass chunk
ACH = 2048           # ACT pass chunk
GCH = 2048           # gpsimd chunk
NRED = float(FB)

_cached = {}


def _host_params(delta_t, matrices):
    dt = np.clip(delta_t.astype(np.float64), 0, 6)[:, 0]
    m = matrices.reshape(NL, C).astype(np.float64)
    alpha = np.concatenate([[1.0], np.cumprod(1.0 - dt)])
    mtil = m / alpha[:NL, None]
    cc = dt / alpha[1:]
    g0 = 1.0 + mtil[0]
    dmt = mtil[1:] - mtil[:-1]                     # [29, 256]
    gfin = 1.0 - alpha[NL] * mtil[NL - 1]
    epst = EPS / alpha[:NL] ** 2
    n2eps = NRED * NRED * epst
    sixc = 6.0 * cc
    return dt, alpha, mtil, cc, g0, dmt, gfin, n2eps, sixc


def _build_program(sixc, n2eps, alpha_l):
    import concourse.tile as tile
    from concourse import bacc, mybir

    f32 = mybir.dt.float32
    bf16 = mybir.dt.bfloat16
    Alu = mybir.AluOpType
    Act = mybir.ActivationFunctionType

    nc = bacc.Bacc("TRN2", target_bir_lowering=False, debug=False,
                   num_devices=NCORES)
    x1_d = nc.dram_tensor("x1", [P, FB], f32, kind="ExternalInput").ap()
    # 29 flat bf16 rows (dmt), each 256-pattern tiled to 1024
    mtb_d = nc.dram_tensor("mtb", [P, 29 * 1024], bf16, kind="ExternalInput").ap()
    # 2 flat f32 rows: g0, gfin
    mtf_d = nc.dram_tensor("mtf", [P, 2 * 1024], f32, kind="ExternalInput").ap()
    ctab_d = nc.dram_tensor("ctab", [P, 3 * NL], f32, kind="ExternalInput").ap()
    id_d = nc.dram_tensor("ident", [P, P], bf16, kind="ExternalInput").ap()
    out_d = nc.dram_tensor("out", [P, FB], f32, kind="ExternalOutput").ap()

    with tile.TileContext(nc) as tc:
        with (
            tc.tile_pool(name="big", bufs=1) as big,
            tc.tile_pool(name="upool", bufs=2) as upool,
            tc.tile_pool(name="jpool", bufs=2) as jpool,
            tc.tile_pool(name="wpool", bufs=2) as wpool,
            tc.tile_pool(name="apool", bufs=2) as apool,
            tc.tile_pool(name="spool", bufs=3) as spool,
            tc.tile_pool(name="dpool", bufs=3) as dpool,
            tc.tile_pool(name="ppro", bufs=2) as ppro,
            tc.tile_pool(name="pp", bufs=1, space="PSUM") as pp,
        ):
            x1t = big.tile([P, FB], f32, name="x1t")
            x1b = big.tile([P, FB], bf16, name="x1b")
            pbb = [big.tile([P, FB], bf16, name="pbb0"),
                   big.tile([P, FB], bf16, name="pbb1")]
            ct = big.tile([P, 3 * NL], f32, name="ct")
            tI = big.tile([P, P], bf16, name="tI")
            g0r = big.tile([P, 1024], f32, name="g0r")
            gfr = big.tile([P, 1024], f32, name="gfr")
            pfin = big.tile([P, FB], f32, name="pfin")
            zp = pp.tile([P, FB], f32, name="zp")

            def sl(i, w):
                return slice(i * w, (i + 1) * w)

            # ---- input DMAs
            for chi in range(4):
                nc.sync.dma_start(x1t[:, sl(chi, 1024)], x1_d[:, sl(chi, 1024)])
            nc.sync.dma_start(ct[:], ctab_d)
            nc.sync.dma_start(tI[:], id_d)
            nc.sync.dma_start(g0r[:], mtf_d[:, 0:1024])
            nc.sync.dma_start(gfr[:], mtf_d[:, 1024:2048])
            mrow = dpool.tile([P, 1024], bf16, name="mrow_p0", tag="dmb")
            nc.sync.dma_start(mrow[:], mtb_d[:, 0:1024])

            # ---- prologue: per-chunk pipeline  x1 -> z0 -> (zhi, zlo) -> PE
            Sacc0 = apool.tile([P, 4], f32, name="Sacc_p", tag="Sacc4")
            for chi in range(4):
                z0c = ppro.tile([P, 1024], f32, name=f"z0_{chi}", tag="z0")
                nc.vector.scalar_tensor_tensor(z0c[:], x1t[:, sl(chi, 1024)],
                                               0.0, g0r[:], op0=Alu.bypass,
                                               op1=Alu.mult,
                                               accum_out=Sacc0[:, chi:chi + 1])
                zhic = ppro.tile([P, 1024], bf16, name=f"zhi{chi}", tag="zh")
                nc.vector.tensor_copy(zhic[:], z0c[:])
                zloc = ppro.tile([P, 1024], bf16, name=f"zlo{chi}", tag="zl")
                nc.vector.scalar_tensor_tensor(zloc[:], z0c[:], 0.0, zhic[:],
                                               op0=Alu.bypass, op1=Alu.subtract)
                for b2 in range(2):
                    b = 2 * chi + b2
                    nc.tensor.matmul(zp[:, sl(b, BANK)], tI[:],
                                     zhic[:, sl(b2, BANK)], start=True, stop=True)
                    nc.tensor.matmul(zp[:, sl(b, BANK)], tI[:],
                                     zloc[:, sl(b2, BANK)], start=False, stop=True)
                # (hi+lo seed: near-fp32-exact initial state)
            SSacc = apool.tile([P, 2], f32, name="SSacc_p", tag="SSacc")
            for q in range(2):
                jt = jpool.tile([P, ACH], f32, name=f"j_p{q}", tag="junk")
                nc.scalar.activation(jt[:], zp[:, sl(q, ACH)], Act.Square,
                                     bias=0.0, scale=1.0,
                                     accum_out=SSacc[:, q:q + 1])
            nc.vector.tensor_copy(x1b[:], x1t[:])
            for g in range(4):
                nc.vector.tensor_tensor(pbb[0][:, sl(g, 1024)],
                                        x1b[:, sl(g, 1024)],
                                        mrow[:], op=Alu.mult)

            Scur = None
            s2e = None
            san = None
            for l in range(NL):
                # ---- per-layer BN coefficients
                # DVE head: SSg -> t1 -> v -> rc ; ACT tail: rs -> a/an2 -> bb
                if l == 0:
                    Scur = spool.tile([P, 1], f32, name="S0", tag="S")
                    nc.vector.tensor_reduce(Scur[:], Sacc0[:],
                                            axis=mybir.AxisListType.X, op=Alu.add)
                    s2e = spool.tile([P, 1], f32, name="s2e0", tag="s2e")
                    nc.vector.tensor_scalar(s2e[:], Scur[:], Scur[:],
                                            -float(n2eps[0]),
                                            op0=Alu.mult, op1=Alu.add)
                    san = spool.tile([P, 1], f32, name="san0", tag="san")
                    nc.vector.tensor_scalar(san[:], Scur[:], ct[:, NL:NL + 1],
                                            None, op0=Alu.mult)
                # chain: SSg -> v -> rc -> sqrt -> {a, bb}
                SSg = spool.tile([P, 1], f32, name=f"SSg{l}", tag="SSg")
                nc.vector.tensor_reduce(SSg[:], SSacc[:],
                                        axis=mybir.AxisListType.X, op=Alu.add)
                v = spool.tile([P, 1], f32, name=f"v{l}", tag="v")
                nc.vector.tensor_scalar(v[:], SSg[:], NRED, s2e[:],
                                        op0=Alu.mult, op1=Alu.subtract)
                rc = spool.tile([P, 1], f32, name=f"rc{l}", tag="rc")
                nc.vector.reciprocal(rc[:], v[:])
                rs = spool.tile([P, 1], f32, name=f"rs{l}", tag="rs")
                nc.scalar.activation(rs[:], rc[:], Act.Sqrt)
                a = spool.tile([P, 1], f32, name=f"a{l}", tag="a")
                nc.vector.tensor_scalar(a[:], rs[:], ct[:, l:l + 1], None,
                                        op0=Alu.mult)
                bb = spool.tile([P, 1], f32, name=f"bb{l}", tag="bb")
                nc.vector.tensor_scalar(bb[:], rs[:], san[:],
                                        ct[:, 2 * NL + l:2 * NL + l + 1],
                                        op0=Alu.mult, op1=Alu.add)
                # produce P_{l+1} on DVE (bf16 2x mode) while ACT runs A
                if l < NL - 2:
                    mrow = dpool.tile([P, 1024], bf16, name=f"mr{l}", tag="dmb")
                    nc.sync.dma_start(mrow[:], mtb_d[:, sl(l + 1, 1024)])
                    for g in range(4):
                        nc.vector.tensor_tensor(pbb[(l + 1) % 2][:, sl(g, 1024)],
                                                x1b[:, sl(g, 1024)],
                                                mrow[:], op=Alu.mult)

                if NL - 4 <= l:
                    g = l - (NL - 4)
                    nc.vector.tensor_tensor(pfin[:, sl(g, 1024)],
                                            x1t[:, sl(g, 1024)],
                                            gfr[:], op=Alu.mult)
                # ---- A: u = Relu(a*z + b) from PSUM (4 x 1024)
                us = []
                for q in range(4):
                    u = upool.tile([P, 1024], f32, name=f"u{l}_{q}", tag=f"u{q}")
                    nc.scalar.activation(u[:], zp[:, sl(q, 1024)], Act.Relu,
                                         bias=bb[:], scale=a[:])
                    us.append(u)
                # ---- wp = min(u, 6c) + P_l -> bf16 (accum -> sum), then PE adds
                WPC = [(0, 1024), (1024, 1024), (2048, 1024),
                       (3072, 512), (3584, 512)]
                Wacc = apool.tile([P, len(WPC)], f32, name=f"Wacc{l}", tag="Wacc")
                for ch, (off, wid) in enumerate(WPC):
                    wb = wpool.tile([P, wid], bf16, name=f"w{l}_{ch}", tag=f"w{ch}")
                    uin = us[off // 1024][:, off % 1024:off % 1024 + wid]
                    if l < NL - 1:
                        nc.vector.scalar_tensor_tensor(
                            wb[:], uin, float(sixc[l]),
                            pbb[l % 2][:, off:off + wid],
                            op0=Alu.min, op1=Alu.add,
                            accum_out=Wacc[:, ch:ch + 1])
                    else:
                        nc.vector.tensor_scalar(wb[:], uin, float(sixc[l]), 0.0,
                                                op0=Alu.min, op1=Alu.add)
                    for b2 in range(wid // BANK):
                        b = (off + b2 * BANK) // BANK
                        nc.tensor.matmul(zp[:, sl(b, BANK)], tI[:],
                                         wb[:, sl(b2, BANK)],
                                         start=False, stop=True)
                if l < NL - 1:
                    # ---- Q: SS of new state, chunks trail the PE pipeline
                    QC = [(0, 2048), (2048, 2048)]
                    SSacc = apool.tile([P, len(QC)], f32, name=f"SSacc{l}",
                                       tag="SSacc")
                    for qi, (off, wid) in enumerate(QC):
                        jt = jpool.tile([P, wid], f32, name=f"j{l}_{qi}",
                                        tag=f"junk{qi}")
                        nc.scalar.activation(jt[:], zp[:, off:off + wid],
                                             Act.Square, bias=0.0, scale=1.0,
                                             accum_out=SSacc[:, qi:qi + 1])
                    # ---- S tracking (off critical path): S += sum(wp)
                    Wsum = spool.tile([P, 1], f32, name=f"Ws{l}", tag="Ws")
                    nc.vector.tensor_reduce(Wsum[:], Wacc[:],
                                            axis=mybir.AxisListType.X, op=Alu.add)
                    Snew = spool.tile([P, 1], f32, name=f"S{l + 1}", tag="S")
                    nc.vector.tensor_scalar(Snew[:], Wsum[:], Scur[:], None,
                                            op0=Alu.add)
                    Scur = Snew
                    s2e = spool.tile([P, 1], f32, name=f"s2e{l + 1}", tag="s2e")
                    nc.vector.tensor_scalar(s2e[:], Snew[:], Snew[:],
                                            -float(n2eps[l + 1]),
                                            op0=Alu.mult, op1=Alu.add)
                    san = spool.tile([P, 1], f32, name=f"san{l + 1}", tag="san")
                    nc.vector.tensor_scalar(san[:], Snew[:],
                                            ct[:, NL + l + 1:NL + l + 2],
                                            None, op0=Alu.mult)

            # ---- epilogue: out = alpha_L * z + gfin * x1
            for chi in range(4):
                o = upool.tile([P, 1024], f32, name=f"o{chi}", tag="obuf")
                nc.vector.scalar_tensor_tensor(o[:], zp[:, sl(chi, 1024)],
                                               float(alpha_l),
                                               pfin[:, sl(chi, 1024)],
                                               op0=Alu.mult, op1=Alu.add)
                nc.sync.dma_start(out_d[:, sl(chi, 1024)], o[:])

    nc.compile()
    return nc


def _get_nc(sixc, n2eps, alpha_l):
    key = (tuple(np.asarray(sixc, np.float64)),
           tuple(np.asarray(n2eps, np.float64)), float(alpha_l))
    if key not in _cached:
        _cached[key] = _build_program(sixc, n2eps, alpha_l)
    return _cached[key]


def _prepare_in_maps(x, delta_t, matrices, gamma, beta):
    dt, alpha, mtil, cc, g0, dmt, gfin, n2eps, sixc = _host_params(delta_t, matrices)

    reps = 1024 // C
    mtb = np.tile(dmt.astype(np.float32), (1, reps)).reshape(1, 29 * 1024)
    mtb_b = np.broadcast_to(mtb.astype(ml_dtypes.bfloat16), (P, 29 * 1024)).copy()
    mtf = np.tile(np.stack([g0, gfin]).astype(np.float32), (1, reps)).reshape(1, 2 * 1024)
    mtf_b = np.broadcast_to(mtf.astype(np.float32), (P, 2 * 1024)).copy()
    ident = np.eye(P, dtype=ml_dtypes.bfloat16)

    g64 = gamma.astype(np.float64)
    b64 = beta.astype(np.float64)
    x1_full = x.reshape(B, C, HW).transpose(2, 0, 1)   # [HW, B, C]

    in_maps = []
    for k in range(NCORES):
        slc = slice(k * P, (k + 1) * P)
        cgN = (cc[:, None] * g64[None, slc] * NRED).T.astype(np.float32)
        cgneg = (-cc[:, None] * g64[None, slc]).T.astype(np.float32)
        cb = (cc[:, None] * b64[None, slc]).T.astype(np.float32)
        ctab = np.ascontiguousarray(np.concatenate([cgN, cgneg, cb], axis=1))
        x1s = np.ascontiguousarray(x1_full[slc]).reshape(P, FB).astype(np.float32)
        in_maps.append({"x1": x1s, "mtb": mtb_b, "mtf": mtf_b, "ctab": ctab,
                        "ident": ident})
    return in_maps, (sixc, n2eps, alpha[NL])


def _gather(results):
    out = np.empty((HW, B, C), dtype=np.float32)
    for k in range(NCORES):
        out[k * P:(k + 1) * P] = results[k]["out"].reshape(P, B, C)
    return np.ascontiguousarray(out.transpose(1, 2, 0).reshape(B, C, H, W))


def _run(trace, **inputs):
    from concourse.bass_utils import run_bass_kernel_spmd
    in_maps, (sixc, n2eps, alpha_l) = _prepare_in_maps(
        np.asarray(inputs["x"]), np.asarray(inputs["delta_t"]),
        np.asarray(inputs["matrices"]), np.asarray(inputs["gamma"]),
        np.asarray(inputs["beta"]))
    nc = _get_nc(sixc, n2eps, alpha_l)
    res = run_bass_kernel_spmd(nc, in_maps, core_ids=list(range(NCORES)),
                               trace=trace)
    return _gather(res.results), res


def kernel(**inputs) -> np.ndarray:
    out, _ = _run(False, **inputs)
    return out


def kernel_traced(**inputs):
    """Returns (output, BassKernelResults) with exec_time_ns populated."""
    return _run(True, **inputs)


# revision 28
# speedup vs baseline: 1.0207x; 1.0145x over previous
"""Trainium2 Bass kernel: AdaptiveDiscretizedNeuralODE (30-step scan with
training-mode BatchNorm over the HW=1024 channel axis, ReLU6, residual).

Design:
 - Channel-shard the 1024 BN channels over 8 NeuronCores -> 128 channels/core
   = the 128 SBUF partitions. BN stats/affine/ReLU6/residual are per-channel,
   so the 8 cores are fully independent (no collectives).
 - Scale folding: BN is invariant under per-layer positive rescaling of its
   input (eps adjusted by 1/alpha_l^2), so the recurrence
       y_{l+1} = (1-dt_l) y_l + dt_l relu6(BN(y_l + m_l*x1))
   becomes, with z_l = y_l/alpha_l + (m_l/alpha_l)*x1:
       z_{l+1} = z_l + min(relu(a_l z_l + b_l), 6 c_l) + (mtil_{l+1}-mtil_l)*x1
   where a_l, b_l fold the BN affine, gamma/beta, and c_l = dt_l/alpha_{l+1}.
 - The state z stays resident in PSUM ([128, 4096] fp32 = all 8 banks) for all
   30 steps; every add into the state is an identity-matmul accumulation on
   the otherwise-idle TensorEngine (PSUM accumulates in fp32 exactly).
 - Steady-state engine split per layer (~10.8 us, DVE and ACT both ~95% busy):
     ACT:  u = Relu(a*z + b)  (4x1024, reads PSUM, per-partition scale/bias)
           Square(z_new) accum -> SS  (2x2048, reads PSUM)
           sqrt for rstd
     DVE:  wp = min(u, 6c) + P_l -> bf16, accum -> sum(wp)   (fused STT)
           P_{l+1} = x1_bf16 * dmtil_{l+1} (bf16 2x-mode TTs, flat tables)
           small [128,1] stats ops (cheap tensor_scalar forms only)
     PE:   z += I @ wp  (8 bank matmuls, bf16 rhs, fp32 accumulate)
 - S (sum) is tracked via the wp accumulators (S += sum(wp)); SS is measured
   from PSUM each layer. The initial state is seeded as a hi+lo bf16 pair
   (near-fp32-exact). Per-layer scalars (6c_l, N^2 eps_l, alpha_L) are baked
   as immediates; the neuron compile cache makes that one-time per input set.
 - Host side does layout only: reshape/transpose of x into per-core shards,
   parameter-table construction from delta_t/matrices/gamma/beta, and the
   inverse layout transform on the output.
"""
import numpy as np
import ml_dtypes

B, C, H, W = 16, 256, 32, 32
HW = H * W
NL = 30
EPS = 1e-5
NCORES = 8
P = 128
FB = B * C           # 4096 free elements per partition
BANK = 512           # psum bank = 512 fp32
NBANK = FB // BANK   # 8
WCH = 1024           # w-pass chunk
ACH = 2048           # ACT pass chunk
GCH = 2048           # gpsimd chunk
NRED = float(FB)

_cached = {}


def _host_params(delta_t, matrices):
    dt = np.clip(delta_t.astype(np.float64), 0, 6)[:, 0]
    m = matrices.reshape(NL, C).astype(np.float64)
    alpha = np.concatenate([[1.0], np.cumprod(1.0 - dt)])
    mtil = m / alpha[:NL, None]
    cc = dt / alpha[1:]
    g0 = 1.0 + mtil[0]
    dmt = mtil[1:] - mtil[:-1]                     # [29, 256]
    gfin = 1.0 - alpha[NL] * mtil[NL - 1]
    epst = EPS / alpha[:NL] ** 2
    n2eps = NRED * NRED * epst
    sixc = 6.0 * cc
    return dt, alpha, mtil, cc, g0, dmt, gfin, n2eps, sixc


def _build_program(sixc, n2eps, alpha_l):
    import concourse.tile as tile
    from concourse import bacc, mybir

    f32 = mybir.dt.float32
    bf16 = mybir.dt.bfloat16
    Alu = mybir.AluOpType
    Act = mybir.ActivationFunctionType

    nc = bacc.Bacc("TRN2", target_bir_lowering=False, debug=False,
                   num_devices=NCORES)
    x1_d = nc.dram_tensor("x1", [P, FB], f32, kind="ExternalInput").ap()
    # 29 flat bf16 rows (dmt), each 256-pattern tiled to 1024
    mtb_d = nc.dram_tensor("mtb", [P, 29 * 1024], bf16, kind="ExternalInput").ap()
    # 2 flat f32 rows: g0, gfin
    mtf_d = nc.dram_tensor("mtf", [P, 2 * 1024], f32, kind="ExternalInput").ap()
    ctab_d = nc.dram_tensor("ctab", [P, 3 * NL], f32, kind="ExternalInput").ap()
    id_d = nc.dram_tensor("ident", [P, P], bf16, kind="ExternalInput").ap()
    out_d = nc.dram_tensor("out", [P, FB], f32, kind="ExternalOutput").ap()

    with tile.TileContext(nc) as tc:
        with (
            tc.tile_pool(name="big", bufs=1) as big,
            tc.tile_pool(name="upool", bufs=2) as upool,
            tc.tile_pool(name="jpool", bufs=2) as jpool,
            tc.tile_pool(name="wpool", bufs=2) as wpool,
            tc.tile_pool(name="apool", bufs=2) as apool,
            tc.tile_pool(name="spool", bufs=3) as spool,
            tc.tile_pool(name="dpool", bufs=3) as dpool,
            tc.tile_pool(name="ppro", bufs=2) as ppro,
            tc.tile_pool(name="pp", bufs=1, space="PSUM") as pp,
        ):
            x1t = big.tile([P, FB], f32, name="x1t")
            x1b = big.tile([P, FB], bf16, name="x1b")
            pbb = [big.tile([P, FB], bf16, name="pbb0"),
                   big.tile([P, FB], bf16, name="pbb1")]
            ct = big.tile([P, 3 * NL], f32, name="ct")
            tI = big.tile([P, P], bf16, name="tI")
            g0r = big.tile([P, 1024], f32, name="g0r")
            gfr = big.tile([P, 1024], f32, name="gfr")
            pfin = big.tile([P, FB], f32, name="pfin")
            zp = pp.tile([P, FB], f32, name="zp")

            def sl(i, w):
                return slice(i * w, (i + 1) * w)

            # ---- input DMAs
            for chi in range(4):
                nc.sync.dma_start(x1t[:, sl(chi, 1024)], x1_d[:, sl(chi, 1024)])
            nc.sync.dma_start(ct[:], ctab_d)
            nc.sync.dma_start(tI[:], id_d)
            nc.sync.dma_start(g0r[:], mtf_d[:, 0:1024])
            nc.sync.dma_start(gfr[:], mtf_d[:, 1024:2048])
            mrow = dpool.tile([P, 1024], bf16, name="mrow_p0", tag="dmb")
            nc.sync.dma_start(mrow[:], mtb_d[:, 0:1024])

            # ---- prologue: per-chunk pipeline  x1 -> z0 -> (zhi, zlo) -> PE
            Sacc0 = apool.tile([P, 4], f32, name="Sacc_p", tag="Sacc4")
            for chi in range(4):
                z0c = ppro.tile([P, 1024], f32, name=f"z0_{chi}", tag="z0")
                nc.vector.scalar_tensor_tensor(z0c[:], x1t[:, sl(chi, 1024)],
                                               0.0, g0r[:], op0=Alu.bypass,
                                               op1=Alu.mult,
                                               accum_out=Sacc0[:, chi:chi + 1])
                zhic = ppro.tile([P, 1024], bf16, name=f"zhi{chi}", tag="zh")
                nc.vector.tensor_copy(zhic[:], z0c[:])
                zloc = ppro.tile([P, 1024], bf16, name=f"zlo{chi}", tag="zl")
                nc.vector.scalar_tensor_tensor(zloc[:], z0c[:], 0.0, zhic[:],
                                               op0=Alu.bypass, op1=Alu.subtract)
                for b2 in range(2):
                    b = 2 * chi + b2
                    nc.tensor.matmul(zp[:, sl(b, BANK)], tI[:],
                                     zhic[:, sl(b2, BANK)], start=True, stop=True)
                    nc.tensor.matmul(zp[:, sl(b, BANK)], tI[:],
                                     zloc[:, sl(b2, BANK)], start=False, stop=True)
                # (hi+lo seed: near-fp32-exact initial state)
            SSacc = apool.tile([P, 2], f32, name="SSacc_p", tag="SSacc")
            for q in range(2):
                jt = jpool.tile([P, ACH], f32, name=f"j_p{q}", tag="junk")
                nc.scalar.activation(jt[:], zp[:, sl(q, ACH)], Act.Square,
                                     bias=0.0, scale=1.0,
                                     accum_out=SSacc[:, q:q + 1])
            nc.vector.tensor_copy(x1b[:], x1t[:])
            for g in range(4):
                nc.vector.tensor_tensor(pbb[0][:, sl(g, 1024)],
                                        x1b[:, sl(g, 1024)],
                                        mrow[:], op=Alu.mult)

            Scur = None
            s2e = None
            san = None
            for l in range(NL):
                # ---- per-layer BN coefficients
                # DVE head: SSg -> t1 -> v -> rc ; ACT tail: rs -> a/an2 -> bb
                if l == 0:
                    Scur = spool.tile([P, 1], f32, name="S0", tag="S")
                    nc.vector.tensor_reduce(Scur[:], Sacc0[:],
                                            axis=mybir.AxisListType.X, op=Alu.add)
                    s2e = spool.tile([P, 1], f32, name="s2e0", tag="s2e")
                    nc.vector.tensor_scalar(s2e[:], Scur[:], Scur[:],
                                            -float(n2eps[0]),
                                            op0=Alu.mult, op1=Alu.add)
                    san = spool.tile([P, 1], f32, name="san0", tag="san")
                    nc.vector.tensor_scalar(san[:], Scur[:], ct[:, NL:NL + 1],
                                            None, op0=Alu.mult)
                # chain: v = N*SSb + ta  (ta = N*SSa - s2e, computed off-chain)
                ta = spool.tile([P, 1], f32, name=f"ta{l}", tag="ta")
                nc.vector.tensor_scalar(ta[:], SSacc[:, 0:1], NRED, s2e[:],
                                        op0=Alu.mult, op1=Alu.subtract)
                v = spool.tile([P, 1], f32, name=f"v{l}", tag="v")
                nc.vector.tensor_scalar(v[:], SSacc[:, 1:2], NRED, ta[:],
                                        op0=Alu.mult, op1=Alu.add)
                rc = spool.tile([P, 1], f32, name=f"rc{l}", tag="rc")
                nc.vector.reciprocal(rc[:], v[:])
                rs = spool.tile([P, 1], f32, name=f"rs{l}", tag="rs")
                nc.scalar.activation(rs[:], rc[:], Act.Sqrt)
                a = spool.tile([P, 1], f32, name=f"a{l}", tag="a")
                nc.vector.tensor_scalar(a[:], rs[:], ct[:, l:l + 1], None,
                                        op0=Alu.mult)
                bb = spool.tile([P, 1], f32, name=f"bb{l}", tag="bb")
                nc.vector.tensor_scalar(bb[:], rs[:], san[:],
                                        ct[:, 2 * NL + l:2 * NL + l + 1],
                                        op0=Alu.mult, op1=Alu.add)
                # produce P_{l+1} on DVE (bf16 2x mode) while ACT runs A
                if l < NL - 2:
                    mrow = dpool.tile([P, 1024], bf16, name=f"mr{l}", tag="dmb")
                    nc.sync.dma_start(mrow[:], mtb_d[:, sl(l + 1, 1024)])
                    for g in range(4):
                        nc.vector.tensor_tensor(pbb[(l + 1) % 2][:, sl(g, 1024)],
                                                x1b[:, sl(g, 1024)],
                                                mrow[:], op=Alu.mult)

                if NL - 4 <= l:
                    g = l - (NL - 4)
                    nc.vector.tensor_tensor(pfin[:, sl(g, 1024)],
                                            x1t[:, sl(g, 1024)],
                                            gfr[:], op=Alu.mult)
                # ---- A: u = Relu(a*z + b) from PSUM (4 x 1024)
                us = []
                for q in range(4):
                    u = upool.tile([P, 1024], f32, name=f"u{l}_{q}", tag=f"u{q}")
                    nc.scalar.activation(u[:], zp[:, sl(q, 1024)], Act.Relu,
                                         bias=bb[:], scale=a[:])
                    us.append(u)
                # ---- wp = min(u, 6c) + P_l -> bf16 (accum -> sum), then PE adds
                WPC = [(0, 1024), (1024, 1024), (2048, 1024),
                       (3072, 512), (3584, 512)]
                Wacc = apool.tile([P, len(WPC)], f32, name=f"Wacc{l}", tag="Wacc")
                for ch, (off, wid) in enumerate(WPC):
                    wb = wpool.tile([P, wid], bf16, name=f"w{l}_{ch}", tag=f"w{ch}")
                    uin = us[off // 1024][:, off % 1024:off % 1024 + wid]
                    if l < NL - 1:
                        nc.vector.scalar_tensor_tensor(
                            wb[:], uin, float(sixc[l]),
                            pbb[l % 2][:, off:off + wid],
                            op0=Alu.min, op1=Alu.add,
                            accum_out=Wacc[:, ch:ch + 1])
                    else:
                        nc.vector.tensor_scalar(wb[:], uin, float(sixc[l]), 0.0,
                                                op0=Alu.min, op1=Alu.add)
                    for b2 in range(wid // BANK):
                        b = (off + b2 * BANK) // BANK
                        nc.tensor.matmul(zp[:, sl(b, BANK)], tI[:],
                                         wb[:, sl(b2, BANK)],
                                         start=False, stop=True)
                if l < NL - 1:
                    # ---- Q: SS of new state, chunks trail the PE pipeline
                    QC = [(0, 2048), (2048, 2048)]
                    SSacc = apool.tile([P, len(QC)], f32, name=f"SSacc{l}",
                                       tag="SSacc")
                    for qi, (off, wid) in enumerate(QC):
                        jt = jpool.tile([P, wid], f32, name=f"j{l}_{qi}",
                                        tag=f"junk{qi}")
                        nc.scalar.activation(jt[:], zp[:, off:off + wid],
                                             Act.Square, bias=0.0, scale=1.0,
                                             accum_out=SSacc[:, qi:qi + 1])
                    # ---- S tracking (off critical path): S += sum(wp)
                    Wsum = spool.tile([P, 1], f32, name=f"Ws{l}", tag="Ws")
                    nc.vector.tensor_reduce(Wsum[:], Wacc[:],
                                            axis=mybir.AxisListType.X, op=Alu.add)
                    Snew = spool.tile([P, 1], f32, name=f"S{l + 1}", tag="S")
                    nc.vector.tensor_scalar(Snew[:], Wsum[:], Scur[:], None,
                                            op0=Alu.add)
                    Scur = Snew
                    s2e = spool.tile([P, 1], f32, name=f"s2e{l + 1}", tag="s2e")
                    nc.vector.tensor_scalar(s2e[:], Snew[:], Snew[:],
                                            -float(n2eps[l + 1]),
                                            op0=Alu.mult, op1=Alu.add)
                    san = spool.tile([P, 1], f32, name=f"san{l + 1}", tag="san")
                    nc.vector.tensor_scalar(san[:], Snew[:],
                                            ct[:, NL + l + 1:NL + l + 2],
                                            None, op0=Alu.mult)

            # ---- epilogue: out = alpha_L * z + gfin * x1
            for chi in range(4):
                o = upool.tile([P, 1024], f32, name=f"o{chi}", tag=f"u{chi}")
                nc.vector.scalar_tensor_tensor(o[:], zp[:, sl(chi, 1024)],
                                               float(alpha_l),
                                               pfin[:, sl(chi, 1024)],
                                               op0=Alu.mult, op1=Alu.add)
                nc.sync.dma_start(out_d[:, sl(chi, 1024)], o[:])

    nc.compile()
    return nc


def _get_nc(sixc, n2eps, alpha_l):
    key = (tuple(np.asarray(sixc, np.float64)),
           tuple(np.asarray(n2eps, np.float64)), float(alpha_l))
    if key not in _cached:
        _cached[key] = _build_program(sixc, n2eps, alpha_l)
    return _cached[key]


def _prepare_in_maps(x, delta_t, matrices, gamma, beta):
    dt, alpha, mtil, cc, g0, dmt, gfin, n2eps, sixc = _host_params(delta_t, matrices)

    reps = 1024 // C
    mtb = np.tile(dmt.astype(np.float32), (1, reps)).reshape(1, 29 * 1024)
    mtb_b = np.broadcast_to(mtb.astype(ml_dtypes.bfloat16), (P, 29 * 1024)).copy()
    mtf = np.tile(np.stack([g0, gfin]).astype(np.float32), (1, reps)).reshape(1, 2 * 1024)
    mtf_b = np.broadcast_to(mtf.astype(np.float32), (P, 2 * 1024)).copy()
    ident = np.eye(P, dtype=ml_dtypes.bfloat16)

    g64 = gamma.astype(np.float64)
    b64 = beta.astype(np.float64)
    x1_full = x.reshape(B, C, HW).transpose(2, 0, 1)   # [HW, B, C]

    in_maps = []
    for k in range(NCORES):
        slc = slice(k * P, (k + 1) * P)
        cgN = (cc[:, None] * g64[None, slc] * NRED).T.astype(np.float32)
        cgneg = (-cc[:, None] * g64[None, slc]).T.astype(np.float32)
        cb = (cc[:, None] * b64[None, slc]).T.astype(np.float32)
        ctab = np.ascontiguousarray(np.concatenate([cgN, cgneg, cb], axis=1))
        x1s = np.ascontiguousarray(x1_full[slc]).reshape(P, FB).astype(np.float32)
        in_maps.append({"x1": x1s, "mtb": mtb_b, "mtf": mtf_b, "ctab": ctab,
                        "ident": ident})
    return in_maps, (sixc, n2eps, alpha[NL])


def _gather(results):
    out = np.empty((HW, B, C), dtype=np.float32)
    for k in range(NCORES):
        out[k * P:(k + 1) * P] = results[k]["out"].reshape(P, B, C)
    return np.ascontiguousarray(out.transpose(1, 2, 0).reshape(B, C, H, W))


def _run(trace, **inputs):
    from concourse.bass_utils import run_bass_kernel_spmd
    in_maps, (sixc, n2eps, alpha_l) = _prepare_in_maps(
        np.asarray(inputs["x"]), np.asarray(inputs["delta_t"]),
        np.asarray(inputs["matrices"]), np.asarray(inputs["gamma"]),
        np.asarray(inputs["beta"]))
    nc = _get_nc(sixc, n2eps, alpha_l)
    res = run_bass_kernel_spmd(nc, in_maps, core_ids=list(range(NCORES)),
                               trace=trace)
    return _gather(res.results), res


def kernel(**inputs) -> np.ndarray:
    out, _ = _run(False, **inputs)
    return out


def kernel_traced(**inputs):
    """Returns (output, BassKernelResults) with exec_time_ns populated."""
    return _run(True, **inputs)


# revision 29
# speedup vs baseline: 1.0247x; 1.0039x over previous
"""Trainium2 Bass kernel: AdaptiveDiscretizedNeuralODE (30-step scan with
training-mode BatchNorm over the HW=1024 channel axis, ReLU6, residual).

Design:
 - Channel-shard the 1024 BN channels over 8 NeuronCores -> 128 channels/core
   = the 128 SBUF partitions. BN stats/affine/ReLU6/residual are per-channel,
   so the 8 cores are fully independent (no collectives).
 - Scale folding: BN is invariant under per-layer positive rescaling of its
   input (eps adjusted by 1/alpha_l^2), so the recurrence
       y_{l+1} = (1-dt_l) y_l + dt_l relu6(BN(y_l + m_l*x1))
   becomes, with z_l = y_l/alpha_l + (m_l/alpha_l)*x1:
       z_{l+1} = z_l + min(relu(a_l z_l + b_l), 6 c_l) + (mtil_{l+1}-mtil_l)*x1
   where a_l, b_l fold the BN affine, gamma/beta, and c_l = dt_l/alpha_{l+1}.
 - The state z stays resident in PSUM ([128, 4096] fp32 = all 8 banks) for all
   30 steps; every add into the state is an identity-matmul accumulation on
   the otherwise-idle TensorEngine (PSUM accumulates in fp32 exactly).
 - Steady-state engine split per layer (~10.8 us, DVE and ACT both ~95% busy):
     ACT:  u = Relu(a*z + b)  (4x1024, reads PSUM, per-partition scale/bias)
           Square(z_new) accum -> SS  (2x2048, reads PSUM)
           sqrt for rstd
     DVE:  wp = min(u, 6c) + P_l -> bf16, accum -> sum(wp)   (fused STT)
           P_{l+1} = x1_bf16 * dmtil_{l+1} (bf16 2x-mode TTs, flat tables)
           small [128,1] stats ops (cheap tensor_scalar forms only)
     PE:   z += I @ wp  (8 bank matmuls, bf16 rhs, fp32 accumulate)
 - S (sum) is tracked via the wp accumulators (S += sum(wp)); SS is measured
   from PSUM each layer. The initial state is seeded as a hi+lo bf16 pair
   (near-fp32-exact). Per-layer scalars (6c_l, N^2 eps_l, alpha_L) are baked
   as immediates; the neuron compile cache makes that one-time per input set.
 - Host side does layout only: reshape/transpose of x into per-core shards,
   parameter-table construction from delta_t/matrices/gamma/beta, and the
   inverse layout transform on the output.
"""
import numpy as np
import ml_dtypes

B, C, H, W = 16, 256, 32, 32
HW = H * W
NL = 30
EPS = 1e-5
NCORES = 8
P = 128
FB = B * C           # 4096 free elements per partition
BANK = 512           # psum bank = 512 fp32
NBANK = FB // BANK   # 8
WCH = 1024           # w-pass chunk
ACH = 2048           # ACT pass chunk
GCH = 2048           # gpsimd chunk
NRED = float(FB)

_cached = {}


def _host_params(delta_t, matrices):
    dt = np.clip(delta_t.astype(np.float64), 0, 6)[:, 0]
    m = matrices.reshape(NL, C).astype(np.float64)
    alpha = np.concatenate([[1.0], np.cumprod(1.0 - dt)])
    mtil = m / alpha[:NL, None]
    cc = dt / alpha[1:]
    g0 = 1.0 + mtil[0]
    dmt = mtil[1:] - mtil[:-1]                     # [29, 256]
    gfin = 1.0 - alpha[NL] * mtil[NL - 1]
    epst = EPS / alpha[:NL] ** 2
    n2eps = NRED * NRED * epst
    sixc = 6.0 * cc
    return dt, alpha, mtil, cc, g0, dmt, gfin, n2eps, sixc


def _build_program(sixc, n2eps, alpha_l):
    import concourse.tile as tile
    from concourse import bacc, mybir

    f32 = mybir.dt.float32
    bf16 = mybir.dt.bfloat16
    Alu = mybir.AluOpType
    Act = mybir.ActivationFunctionType

    nc = bacc.Bacc("TRN2", target_bir_lowering=False, debug=False,
                   num_devices=NCORES)
    x1_d = nc.dram_tensor("x1", [P, FB], f32, kind="ExternalInput").ap()
    # 29 flat bf16 rows (dmt), each 256-pattern tiled to 1024
    mtb_d = nc.dram_tensor("mtb", [P, 29 * 1024], bf16, kind="ExternalInput").ap()
    # 2 flat f32 rows: g0, gfin
    mtf_d = nc.dram_tensor("mtf", [P, 2 * 1024], f32, kind="ExternalInput").ap()
    ctab_d = nc.dram_tensor("ctab", [P, 3 * NL], f32, kind="ExternalInput").ap()
    id_d = nc.dram_tensor("ident", [P, P], bf16, kind="ExternalInput").ap()
    out_d = nc.dram_tensor("out", [P, FB], f32, kind="ExternalOutput").ap()

    with tile.TileContext(nc) as tc:
        with (
            tc.tile_pool(name="big", bufs=1) as big,
            tc.tile_pool(name="upool", bufs=2) as upool,
            tc.tile_pool(name="jpool", bufs=2) as jpool,
            tc.tile_pool(name="wpool", bufs=2) as wpool,
            tc.tile_pool(name="apool", bufs=2) as apool,
            tc.tile_pool(name="spool", bufs=3) as spool,
            tc.tile_pool(name="dpool", bufs=3) as dpool,
            tc.tile_pool(name="ppro", bufs=2) as ppro,
            tc.tile_pool(name="pp", bufs=1, space="PSUM") as pp,
        ):
            x1t = big.tile([P, FB], f32, name="x1t")
            x1b = big.tile([P, FB], bf16, name="x1b")
            pbb = [big.tile([P, FB], bf16, name="pbb0"),
                   big.tile([P, FB], bf16, name="pbb1")]
            ct = big.tile([P, 3 * NL], f32, name="ct")
            tI = big.tile([P, P], bf16, name="tI")
            g0r = big.tile([P, 1024], f32, name="g0r")
            gfr = big.tile([P, 1024], f32, name="gfr")
            pfin = big.tile([P, FB], f32, name="pfin")
            zp = pp.tile([P, FB], f32, name="zp")

            def sl(i, w):
                return slice(i * w, (i + 1) * w)

            # ---- input DMAs (ordered so the z0 chain can start ASAP)
            nc.sync.dma_start(x1t[:, sl(0, 1024)], x1_d[:, sl(0, 1024)])
            nc.sync.dma_start(g0r[:], mtf_d[:, 0:1024])
            for chi in range(1, 4):
                nc.sync.dma_start(x1t[:, sl(chi, 1024)], x1_d[:, sl(chi, 1024)])
            nc.sync.dma_start(tI[:], id_d)
            mrow = dpool.tile([P, 1024], bf16, name="mrow_p0", tag="dmb")
            nc.sync.dma_start(mrow[:], mtb_d[:, 0:1024])
            nc.sync.dma_start(ct[:], ctab_d)
            nc.sync.dma_start(gfr[:], mtf_d[:, 1024:2048])
            # (3) pin the ACT table set (sqrt_and_others holds sqrt/square/
            # relu/identity) so no mid-kernel table switch occurs
            dummy = spool.tile([P, 1], f32, name="dummy_sqrt", tag="rs")
            nc.scalar.activation(dummy[:], g0r[:, 0:1], Act.Sqrt)

            # ---- prologue: per-chunk pipeline  x1 -> z0 -> (zhi, zlo) -> PE
            Sacc0 = apool.tile([P, 4], f32, name="Sacc_p", tag="Sacc4")
            for chi in range(4):
                z0c = ppro.tile([P, 1024], f32, name=f"z0_{chi}", tag="z0")
                nc.vector.scalar_tensor_tensor(z0c[:], x1t[:, sl(chi, 1024)],
                                               0.0, g0r[:], op0=Alu.bypass,
                                               op1=Alu.mult,
                                               accum_out=Sacc0[:, chi:chi + 1])
                zhic = ppro.tile([P, 1024], bf16, name=f"zhi{chi}", tag="zh")
                nc.vector.tensor_copy(zhic[:], z0c[:])
                zloc = ppro.tile([P, 1024], bf16, name=f"zlo{chi}", tag="zl")
                last_zlo = nc.vector.scalar_tensor_tensor(zloc[:], z0c[:], 0.0,
                                                          zhic[:], op0=Alu.bypass,
                                                          op1=Alu.subtract)
                for b2 in range(2):
                    b = 2 * chi + b2
                    nc.tensor.matmul(zp[:, sl(b, BANK)], tI[:],
                                     zhic[:, sl(b2, BANK)], start=True, stop=True)
                    nc.tensor.matmul(zp[:, sl(b, BANK)], tI[:],
                                     zloc[:, sl(b2, BANK)], start=False, stop=True)
                # (hi+lo seed: near-fp32-exact initial state)
            SSacc = apool.tile([P, 2], f32, name="SSacc_p", tag="SSacc")
            for q in range(2):
                jt = jpool.tile([P, ACH], f32, name=f"j_p{q}", tag="junk")
                nc.scalar.activation(jt[:], zp[:, sl(q, ACH)], Act.Square,
                                     bias=0.0, scale=1.0,
                                     accum_out=SSacc[:, q:q + 1])
            from concourse.tile_rust import add_dep_helper
            cast_inst = nc.vector.tensor_copy(x1b[:], x1t[:])
            add_dep_helper(cast_inst.ins, last_zlo.ins, sync=True,
                           reason="x1b cast after seed chain")
            for g in range(4):
                nc.vector.tensor_tensor(pbb[0][:, sl(g, 1024)],
                                        x1b[:, sl(g, 1024)],
                                        mrow[:], op=Alu.mult)

            Scur = None
            s2e = None
            san = None
            for l in range(NL):
                # ---- per-layer BN coefficients
                # DVE head: SSg -> t1 -> v -> rc ; ACT tail: rs -> a/an2 -> bb
                if l == 0:
                    Scur = spool.tile([P, 1], f32, name="S0", tag="S")
                    nc.vector.tensor_reduce(Scur[:], Sacc0[:],
                                            axis=mybir.AxisListType.X, op=Alu.add)
                    s2e = spool.tile([P, 1], f32, name="s2e0", tag="s2e")
                    nc.vector.tensor_scalar(s2e[:], Scur[:], Scur[:],
                                            -float(n2eps[0]),
                                            op0=Alu.mult, op1=Alu.add)
                    san = spool.tile([P, 1], f32, name="san0", tag="san")
                    nc.vector.tensor_scalar(san[:], Scur[:], ct[:, NL:NL + 1],
                                            None, op0=Alu.mult)
                # chain: v = N*SSb + ta  (ta = N*SSa - s2e, computed off-chain)
                ta = spool.tile([P, 1], f32, name=f"ta{l}", tag="ta")
                nc.vector.tensor_scalar(ta[:], SSacc[:, 0:1], NRED, s2e[:],
                                        op0=Alu.mult, op1=Alu.subtract)
                v = spool.tile([P, 1], f32, name=f"v{l}", tag="v")
                nc.vector.tensor_scalar(v[:], SSacc[:, 1:2], NRED, ta[:],
                                        op0=Alu.mult, op1=Alu.add)
                rc = spool.tile([P, 1], f32, name=f"rc{l}", tag="rc")
                nc.vector.reciprocal(rc[:], v[:])
                rs = spool.tile([P, 1], f32, name=f"rs{l}", tag="rs")
                nc.scalar.activation(rs[:], rc[:], Act.Sqrt)
                a = spool.tile([P, 1], f32, name=f"a{l}", tag="a")
                nc.vector.tensor_scalar(a[:], rs[:], ct[:, l:l + 1], None,
                                        op0=Alu.mult)
                bb = spool.tile([P, 1], f32, name=f"bb{l}", tag="bb")
                nc.vector.tensor_scalar(bb[:], rs[:], san[:],
                                        ct[:, 2 * NL + l:2 * NL + l + 1],
                                        op0=Alu.mult, op1=Alu.add)
                # produce P_{l+1} on DVE (bf16 2x mode) while ACT runs A
                if l < NL - 2:
                    mrow = dpool.tile([P, 1024], bf16, name=f"mr{l}", tag="dmb")
                    nc.sync.dma_start(mrow[:], mtb_d[:, sl(l + 1, 1024)])
                    for g in range(4):
                        nc.vector.tensor_tensor(pbb[(l + 1) % 2][:, sl(g, 1024)],
                                                x1b[:, sl(g, 1024)],
                                                mrow[:], op=Alu.mult)

                if NL - 4 <= l:
                    g = l - (NL - 4)
                    nc.vector.tensor_tensor(pfin[:, sl(g, 1024)],
                                            x1t[:, sl(g, 1024)],
                                            gfr[:], op=Alu.mult)
                # ---- A: u = Relu(a*z + b) from PSUM (4 x 1024)
                us = []
                for q in range(4):
                    u = upool.tile([P, 1024], f32, name=f"u{l}_{q}", tag=f"u{q}")
                    nc.scalar.activation(u[:], zp[:, sl(q, 1024)], Act.Relu,
                                         bias=bb[:], scale=a[:])
                    us.append(u)
                # ---- wp = min(u, 6c) + P_l -> bf16 (accum -> sum), then PE adds
                WPC = [(0, 1024), (1024, 1024), (2048, 1024),
                       (3072, 512), (3584, 512)]
                Wacc = apool.tile([P, len(WPC)], f32, name=f"Wacc{l}", tag="Wacc")
                for ch, (off, wid) in enumerate(WPC):
                    wb = wpool.tile([P, wid], bf16, name=f"w{l}_{ch}", tag=f"w{ch}")
                    uin = us[off // 1024][:, off % 1024:off % 1024 + wid]
                    if l < NL - 1:
                        nc.vector.scalar_tensor_tensor(
                            wb[:], uin, float(sixc[l]),
                            pbb[l % 2][:, off:off + wid],
                            op0=Alu.min, op1=Alu.add,
                            accum_out=Wacc[:, ch:ch + 1])
                    else:
                        nc.vector.tensor_scalar(wb[:], uin, float(sixc[l]), 0.0,
                                                op0=Alu.min, op1=Alu.add)
                    for b2 in range(wid // BANK):
                        b = (off + b2 * BANK) // BANK
                        nc.tensor.matmul(zp[:, sl(b, BANK)], tI[:],
                                         wb[:, sl(b2, BANK)],
                                         start=False, stop=True)
                if l < NL - 1:
                    # ---- Q: SS of new state, chunks trail the PE pipeline
                    QC = [(0, 2048), (2048, 2048)]
                    SSacc = apool.tile([P, len(QC)], f32, name=f"SSacc{l}",
                                       tag="SSacc")
                    for qi, (off, wid) in enumerate(QC):
                        jt = jpool.tile([P, wid], f32, name=f"j{l}_{qi}",
                                        tag=f"junk{qi}")
                        nc.scalar.activation(jt[:], zp[:, off:off + wid],
                                             Act.Square, bias=0.0, scale=1.0,
                                             accum_out=SSacc[:, qi:qi + 1])
                    # ---- S tracking (off critical path): S += sum(wp)
                    Wsum = spool.tile([P, 1], f32, name=f"Ws{l}", tag="Ws")
                    nc.vector.tensor_reduce(Wsum[:], Wacc[:],
                                            axis=mybir.AxisListType.X, op=Alu.add)
                    Snew = spool.tile([P, 1], f32, name=f"S{l + 1}", tag="S")
                    nc.vector.tensor_scalar(Snew[:], Wsum[:], Scur[:], None,
                                            op0=Alu.add)
                    Scur = Snew
                    s2e = spool.tile([P, 1], f32, name=f"s2e{l + 1}", tag="s2e")
                    nc.vector.tensor_scalar(s2e[:], Snew[:], Snew[:],
                                            -float(n2eps[l + 1]),
                                            op0=Alu.mult, op1=Alu.add)
                    san = spool.tile([P, 1], f32, name=f"san{l + 1}", tag="san")
                    nc.vector.tensor_scalar(san[:], Snew[:],
                                            ct[:, NL + l + 1:NL + l + 2],
                                            None, op0=Alu.mult)

            # ---- epilogue: out = alpha_L * z + gfin * x1
            for chi in range(4):
                o = upool.tile([P, 1024], f32, name=f"o{chi}", tag=f"u{chi}")
                nc.vector.scalar_tensor_tensor(o[:], zp[:, sl(chi, 1024)],
                                               float(alpha_l),
                                               pfin[:, sl(chi, 1024)],
                                               op0=Alu.mult, op1=Alu.add)
                nc.sync.dma_start(out_d[:, sl(chi, 1024)], o[:])

    nc.compile()
    return nc


def _get_nc(sixc, n2eps, alpha_l):
    key = (tuple(np.asarray(sixc, np.float64)),
           tuple(np.asarray(n2eps, np.float64)), float(alpha_l))
    if key not in _cached:
        _cached[key] = _build_program(sixc, n2eps, alpha_l)
    return _cached[key]


def _prepare_in_maps(x, delta_t, matrices, gamma, beta):
    dt, alpha, mtil, cc, g0, dmt, gfin, n2eps, sixc = _host_params(delta_t, matrices)

    reps = 1024 // C
    mtb = np.tile(dmt.astype(np.float32), (1, reps)).reshape(1, 29 * 1024)
    mtb_b = np.broadcast_to(mtb.astype(ml_dtypes.bfloat16), (P, 29 * 1024)).copy()
    mtf = np.tile(np.stack([g0, gfin]).astype(np.float32), (1, reps)).reshape(1, 2 * 1024)
    mtf_b = np.broadcast_to(mtf.astype(np.float32), (P, 2 * 1024)).copy()
    ident = np.eye(P, dtype=ml_dtypes.bfloat16)

    g64 = gamma.astype(np.float64)
    b64 = beta.astype(np.float64)
    x1_full = x.reshape(B, C, HW).transpose(2, 0, 1)   # [HW, B, C]

    in_maps = []
    for k in range(NCORES):
        slc = slice(k * P, (k + 1) * P)
        cgN = (cc[:, None] * g64[None, slc] * NRED).T.astype(np.float32)
        cgneg = (-cc[:, None] * g64[None, slc]).T.astype(np.float32)
        cb = (cc[:, None] * b64[None, slc]).T.astype(np.float32)
        ctab = np.ascontiguousarray(np.concatenate([cgN, cgneg, cb], axis=1))
        x1s = np.ascontiguousarray(x1_full[slc]).reshape(P, FB).astype(np.float32)
        in_maps.append({"x1": x1s, "mtb": mtb_b, "mtf": mtf_b, "ctab": ctab,
                        "ident": ident})
    return in_maps, (sixc, n2eps, alpha[NL])


def _gather(results):
    out = np.empty((HW, B, C), dtype=np.float32)
    for k in range(NCORES):
        out[k * P:(k + 1) * P] = results[k]["out"].reshape(P, B, C)
    return np.ascontiguousarray(out.transpose(1, 2, 0).reshape(B, C, H, W))


def _run(trace, **inputs):
    from concourse.bass_utils import run_bass_kernel_spmd
    in_maps, (sixc, n2eps, alpha_l) = _prepare_in_maps(
        np.asarray(inputs["x"]), np.asarray(inputs["delta_t"]),
        np.asarray(inputs["matrices"]), np.asarray(inputs["gamma"]),
        np.asarray(inputs["beta"]))
    nc = _get_nc(sixc, n2eps, alpha_l)
    res = run_bass_kernel_spmd(nc, in_maps, core_ids=list(range(NCORES)),
                               trace=trace)
    return _gather(res.results), res


def kernel(**inputs) -> np.ndarray:
    out, _ = _run(False, **inputs)
    return out


def kernel_traced(**inputs):
    """Returns (output, BassKernelResults) with exec_time_ns populated."""
    return _run(True, **inputs)
